# revision 69
# baseline (speedup 1.0000x reference)
"""Trainium2 Bass kernel for GQA attention with RoPE (dense transformer).

Problem: B=2, S=2048, H=2048, 16 query heads / 4 KV heads, head_dim 128,
causal flash-style attention, fused QKV + o_proj.

Sharding (8 cores): (batch, head-group) grid. Core c handles batch c//4 and
head group c%4 (4 query heads + their shared KV head). o_proj is computed as
per-group partials reduced on host (tensor-parallel o_proj input split).

Schedule (~263us vs the 336us fp32r baseline):
  - All SBUF operands and DMA traffic in bf16 (PSUM stays f32); halves HBM
    traffic and dodges the fp32r free-dim<256 4x PE penalty on diagonal
    score tiles.
  - Softmax row-sums off the PE: exp tiles accumulate across k-tiles on the
    Vector engine (bf16), one ones-matmul per (head, q-chunk) reduces
    partitions.
  - V transposed to [tok, d] via DMA xbar, not PE; RoPE's rotate-half is
    folded into the sin multiply (host pre-swaps sin's partition halves).
  - The attention j-loop alone is paced by Scalar EXP (~540ns/tile vs
    ~430ns of PE work per tile), so next-chunk projection blocks and
    previous-chunk o_proj blocks are interleaved into it as PE filler;
    6 blocks run before each j-loop (pre=6) to cover the RoPE and
    rowsum->reciprocal->normalize boundary latencies, and 2 o_proj blocks
    are reserved for after the rowsums.
  - x loaded once as [128, 1024] two-chunk tiles (2KB descriptors; 1KB
    bf16 rows run at half DMA rate), fully resident; weights host-packed
    partition-major and staged on the scalar hwdge queue; outputs
    alternate the sync/scalar hwdge queues.
  - t=0 projections ko-outer with 6 pinned PSUM banks (pq->PO0-3, pk->A,
    pv->OP) so PE consumption tracks x-tile arrival during the
    bandwidth-bound startup; later chunks are block-serial on the A-ring.
  - PSUM: A-ring 3 banks (scores/rowsums/proj blocks) + PO0-3 pinned
    (AV accumulation per head, reused by t=0 pq) + OP (o_proj blocks).
"""
import math

import ml_dtypes
import numpy as np

import concourse.bass as bass
import concourse.mybir as mybir
import concourse.tile as tile
from concourse import bacc
from concourse.bass_utils import run_bass_kernel_spmd

B, S, H = 2, 2048, 2048
NH, KVH, HD = 16, 4, 128
G = 4                 # head groups (= KVH); grid = G x B = 8 cores
GQ = NH // KVH        # query heads per group
QD = GQ * HD          # per-core q dim (512)
KC = H // 128         # contraction chunks for projections (16)
TC = 4                # token chunks of 512
TT = S // 128         # 128-token tiles (16)

F32 = mybir.dt.float32
BF16 = mybir.dt.bfloat16
AF = mybir.ActivationFunctionType

_NC = None


def _emit(nc):
    # weights come host-packed with the SBUF partition dim outermost so each
    # load is one DMA with 128 large contiguous descriptors
    xT = nc.dram_tensor("xT", [H, S], BF16, kind="ExternalInput").ap()
    wqD = nc.dram_tensor("wqD", [128, KC * QD], BF16,
                         kind="ExternalInput").ap()
    wkvD = nc.dram_tensor("wkvD", [128, KC * 2 * HD], BF16,
                          kind="ExternalInput").ap()
    woD = nc.dram_tensor("woD", [128, GQ * H], BF16,
                         kind="ExternalInput").ap()
    cosT = nc.dram_tensor("cosT", [HD, S], F32, kind="ExternalInput").ap()
    sinS = nc.dram_tensor("sinS", [HD, S], F32, kind="ExternalInput").ap()
    bqkv = nc.dram_tensor("bqkv", [128, 6], F32, kind="ExternalInput").ap()
    onesd = nc.dram_tensor("onesd", [128, 128], BF16, kind="ExternalInput").ap()
    outp = nc.dram_tensor("outp", [S, H], BF16, kind="ExternalOutput").ap()

    xT3 = xT.rearrange("(ko p) t -> p ko t", p=128)
    wqD3 = wqD.rearrange("p (ko m) -> p ko m", ko=KC)
    wkvD3 = wkvD.rearrange("p (ko m) -> p ko m", ko=KC)
    woD3 = woD.rearrange("p (ic o) -> p ic o", ic=GQ)

    with tile.TileContext(nc) as tc:
        with (
            tc.tile_pool(name="persist", bufs=1) as pp,
            tc.tile_pool(name="qfp", bufs=2) as pqf,
            tc.tile_pool(name="accp", bufs=1) as pacc,
            tc.tile_pool(name="expp", bufs=1) as pex,
            tc.tile_pool(name="rope", bufs=1) as pr,
            tc.tile_pool(name="outp", bufs=1) as pfo,
            tc.tile_pool(name="psum8", bufs=1, space="PSUM") as ps8,
        ):
            # persistent per-chunk K/V (split per t-chunk to keep dep ranges
            # disjoint between the producing chunk and attention readers)
            kf = [pp.tile([128, 512], BF16, name=f"kf{t}") for t in range(TC)]
            v_sb = [pp.tile([128, 4, HD], BF16, name=f"vsb{t}")
                    for t in range(TC)]
            ofl = pp.tile([128, GQ, S], BF16, name="ofl")

            # ---- constants / weights ----
            bias_sb = pp.tile([128, 6], F32, name="bias")
            nc.gpsimd.dma_start(bias_sb[:, :], bqkv)
            ones_mat = pp.tile([128, 128], BF16, name="ones")
            nc.gpsimd.dma_start(ones_mat[:, :], onesd)
            wq_sb = pp.tile([128, KC, QD], BF16, name="wq")
            wkv_sb = pp.tile([128, KC, 2 * HD], BF16, name="wkv")
            wo_sb = pp.tile([128, GQ, H], BF16, name="wo")
            cos_sb = pp.tile([128, S], F32, name="cos")
            sin_sb = pp.tile([128, S], F32, name="sin")
            # x resident as two-chunk tiles: xp[tp][ko] covers tokens
            # [1024*tp, 1024*tp+1024) for contraction chunk ko
            xp = [[pp.tile([128, 1024], BF16, name=f"x{tp}_{ko}")
                   for ko in range(KC)] for tp in range(2)]

            def oproj_blocks(qc, tail=False):
                """16 emit-callables, one [128tok x 512out] PSUM block each.
                tail: alternate fo evictions across Vector and Scalar --
                Scalar is exp-idle there and the PSUM ring otherwise waits
                ~0.8us per block on Vector evictions."""
                blocks = []
                for tt in range(4 * qc, 4 * qc + 4):
                    for oc in range(4):
                        def blk(tt=tt, oc=oc, tag="OP",
                                evict_scalar=False):
                            tsl = slice(128 * tt, 128 * tt + 128)
                            osl = slice(512 * oc, 512 * oc + 512)
                            pf = ps8.tile([128, 512], F32, tag=tag,
                                          bufs=(1 if tag == "OP" else 3),
                                          name=f"pf_{tt}_{oc}")
                            for ic in range(GQ):
                                nc.tensor.matmul(
                                    pf[:, :], ofl[:, ic, tsl],
                                    wo_sb[:, ic, osl],
                                    start=(ic == 0), stop=(ic == GQ - 1))
                            fo = pfo.tile([128, 512], BF16, tag="fo", bufs=6,
                                          name=f"fo_{tt}_{oc}")
                            if evict_scalar or (
                                    tail and (tt + oc) % 2 == 1):
                                nc.scalar.copy(fo[:, :], pf[:, :])
                            else:
                                nc.vector.tensor_copy(fo[:, :], pf[:, :])
                            # alternate output queues so the drain isn't
                            # single-queue bound at the tail
                            eng = nc.sync if (tt + oc) % 2 == 0 else nc.scalar
                            eng.dma_start(outp[tsl, osl], fo[:, :])
                        blocks.append(blk)
                return blocks

            def rope_evict(t, m, src, qf_t):
                """evict PSUM block m (+bias), apply RoPE, store bf16."""
                tsf = slice(512 * t, 512 * t + 512)
                raw = pr.tile([128, 512], F32, tag="raw", bufs=3,
                              name=f"raw_{t}_{m}")
                bcol = m if m < GQ else 4
                nc.scalar.activation(raw[:, :], src, AF.Identity,
                                     bias=bias_sb[:, bcol:bcol + 1])
                # rotate-half folded into the sin multiply: sin comes
                # host-swapped (halves rolled by 64) so both DVE inputs
                # share a base partition; only the output is shifted
                t1 = pr.tile([128, 512], F32, tag="t1", bufs=2,
                             name=f"t1_{t}_{m}")
                nc.vector.tensor_mul(t1[0:64, :], raw[64:128, :],
                                     sin_sb[64:128, tsf])
                nc.vector.tensor_mul(t1[64:128, :], raw[0:64, :],
                                     sin_sb[0:64, tsf])
                t2 = pr.tile([128, 512], F32, tag="t2", bufs=2,
                             name=f"t2_{t}_{m}")
                nc.vector.tensor_mul(t2[:, :], raw[:, :], cos_sb[:, tsf])
                dst = qf_t[:, m, :] if m < GQ else kf[t][:, :]
                nc.vector.tensor_add(dst, t1[:, :], t2[:, :])

            def v_evict(t, src):
                """evict V (+bias) to bf16, DMA-xbar to [tok, d]."""
                vT_t = pr.tile([128, 512], BF16, tag="vT", bufs=2,
                               name=f"vT_{t}")
                nc.scalar.activation(vT_t[:, :], src, AF.Identity,
                                     bias=bias_sb[:, 5:6])
                for st4 in range(4):
                    nc.sync.dma_start(v_sb[t][:, st4, :],
                                      vT_t[:, 128 * st4:128 * st4 + 128],
                                      transpose=True)

            def proj_blocks(t, qf_t):
                """t>=1: block-serial projection emit-callables (A-ring)."""
                ts = slice(512 * (t % 2), 512 * (t % 2) + 512)
                tp = t // 2
                blocks = []
                for bi in [4, 5, 0, 1, 2, 3]:
                    def blk(bi=bi):
                        pb = ps8.tile([128, 512], F32, tag="A", bufs=3,
                                      name=f"pb_{t}_{bi}")
                        for ko in range(KC):
                            if bi < GQ:
                                w = wq_sb[:, ko, 128 * bi:128 * bi + 128]
                            elif bi == 4:
                                w = wkv_sb[:, ko, 0:HD]
                            else:
                                w = wkv_sb[:, ko, HD:2 * HD]
                            nc.tensor.matmul(pb[:, :], w, xp[tp][ko][:, ts],
                                             start=(ko == 0),
                                             stop=(ko == KC - 1))
                        if bi == 5:
                            v_evict(t, pb[:, :])
                        else:
                            rope_evict(t, bi, pb[:, :], qf_t)
                    blocks.append(blk)
                return blocks

            def attention(qc, qf_t, fillers, reserve=0, pre=0):
                """flash attention for q-chunk qc over k-tiles 0..4qc+3.
                fillers: emit-callables (next-chunk proj blocks + previous
                chunk o_proj blocks) interleaved as PE filler while the
                exp evictions pace the j-loop. `pre` fillers run before the
                j-loop (covers RoPE/normalize latency at the boundary);
                `reserve` fillers are held back to cover the rowsum ->
                reciprocal -> normalize latency at the end."""
                qs = slice(512 * qc, 512 * qc + 512)
                nj = 4 * qc + 4
                acc = pacc.tile([128, GQ, 512], BF16, tag="acc", bufs=2,
                                name=f"acc_{qc}")
                po = [ps8.tile([128, 512], F32, tag=f"PO{h}", bufs=1,
                               name=f"po_{qc}_{h}") for h in range(GQ)]
                ninter = len(fillers) - reserve
                emitted = 0
                while emitted < min(pre, ninter):
                    fillers[emitted]()
                    emitted += 1
                for j in range(nj):
                    off = 0 if j < 4 * qc else 128 * j - 512 * qc
                    n = 512 - off
                    ex = pex.tile([128, GQ, 512], BF16, tag="E", bufs=5,
                                  name=f"ex_{qc}_{j}")
                    for h in range(GQ):
                        ps = ps8.tile([128, 512], F32, tag="A", bufs=3,
                                      name=f"ps_{qc}_{j}_{h}")
                        nc.tensor.matmul(
                            ps[:, 0:n],
                            kf[j // 4][:, 128 * (j % 4):128 * (j % 4) + 128],
                            qf_t[:, h, off:off + n], start=True, stop=True)
                        nc.scalar.activation(ex[:, h, 0:n], ps[:, 0:n],
                                             AF.Exp)
                    if j >= 4 * qc:
                        # zero the strictly-lower (q < k) triangle, all heads
                        nc.gpsimd.affine_select(
                            out=ex[:, :, 0:128], in_=ex[:, :, 0:128],
                            compare_op=mybir.AluOpType.is_ge, fill=0.0,
                            base=0, pattern=[[0, GQ], [1, 128]],
                            channel_multiplier=-1)
                    if j == 0:
                        nc.vector.tensor_copy(acc[:, :, :], ex[:, :, :])
                    else:
                        nc.vector.tensor_add(acc[:, :, off:512],
                                             acc[:, :, off:512],
                                             ex[:, :, 0:n])
                    for h in range(GQ):
                        nc.tensor.matmul(
                            po[h][:, off:off + n],
                            v_sb[j // 4][:, j % 4, :],
                            ex[:, h, 0:n],
                            start=(j == 0), stop=(j == nj - 1))
                    want = max(emitted, (j + 1) * ninter // nj)
                    while emitted < want:
                        fillers[emitted]()
                        emitted += 1
                for h in range(GQ):
                    psum = ps8.tile([128, 512], F32, tag="A", bufs=3,
                                    name=f"psum_{qc}_{h}")
                    nc.tensor.matmul(psum[:, :], ones_mat[:, :],
                                     acc[:, h, :], start=True, stop=True)
                    bc = pr.tile([128, 512], F32, tag="bc", bufs=2,
                                 name=f"bc_{qc}_{h}")
                    nc.vector.reciprocal_approx_fast(bc[:, :], psum[:, :])
                    nc.vector.tensor_mul(ofl[:, h, qs], po[h][:, :],
                                         bc[:, :])
                while emitted < len(fillers):
                    fillers[emitted](evict_scalar=True)
                    emitted += 1

            def proj_chunk0():
                """t=0: ko-outer so PE consumption tracks x-tile arrival
                (x is still streaming in); 6 pinned banks, no attention
                running yet so PO/OP are free."""
                for ko in range(KC):
                    nc.sync.dma_start(xp[0][ko][:, :], xT3[:, ko, 0:1024])
                    if ko == 0:
                        # chunk-0 cos/sin right behind the first x tile so
                        # RoPE(chunk 0) isn't blocked until the bulk loads
                        nc.sync.dma_start(cos_sb[:, 0:512], cosT[:, 0:512])
                        nc.sync.dma_start(sin_sb[:, 0:512], sinS[:, 0:512])
                for kg in range(4):
                    kos = slice(4 * kg, 4 * kg + 4)
                    nc.scalar.dma_start(wq_sb[:, kos, :], wqD3[:, kos, :])
                    nc.scalar.dma_start(wkv_sb[:, kos, :], wkvD3[:, kos, :])
                for cs in [slice(512 * k, 512 * k + 512) for k in (1, 2, 3)]:
                    nc.scalar.dma_start(cos_sb[:, cs], cosT[:, cs])
                    nc.scalar.dma_start(sin_sb[:, cs], sinS[:, cs])

                qf_t = pqf.tile([128, GQ, 512], BF16, tag="qf", name="qf_0")
                pq = [ps8.tile([128, 512], F32, tag=f"PO{m}", bufs=1,
                               name=f"pq{m}_0") for m in range(GQ)]
                pk = ps8.tile([128, 512], F32, tag="A", bufs=3, name="pk_0")
                pv = ps8.tile([128, 512], F32, tag="OP", bufs=1, name="pv_0")
                for ko in range(KC):
                    st = (ko == 0)
                    sp = (ko == KC - 1)
                    xck = xp[0][ko][:, 0:512]
                    nc.tensor.matmul(pk[:, :], wkv_sb[:, ko, 0:HD], xck,
                                     start=st, stop=sp)
                    for m in range(GQ):
                        nc.tensor.matmul(
                            pq[m][:, :],
                            wq_sb[:, ko, 128 * m:128 * m + 128],
                            xck, start=st, stop=sp)
                    nc.tensor.matmul(pv[:, :], wkv_sb[:, ko, HD:2 * HD],
                                     xck, start=st, stop=sp)
                v_evict(0, pv[:, :])
                for m in [GQ, 0, 1, 2, 3]:
                    rope_evict(0, m, pq[m][:, :] if m < GQ else pk[:, :],
                               qf_t)
                return qf_t

            # ============ interleaved projections + attention =============
            qf_tiles = [None] * TC
            qf_tiles[0] = proj_chunk0()
            for t in range(1, TC):
                if t == 1:
                    # second x pair + o_proj weights while proj(1) runs
                    for ko in range(KC):
                        nc.sync.dma_start(xp[1][ko][:, :],
                                          xT3[:, ko, 1024:2048])
                    nc.gpsimd.dma_start(wo_sb[:, :, :], woD3)
                qf_tiles[t] = pqf.tile([128, GQ, 512], BF16, tag="qf",
                                       name=f"qf_{t}")
                fillers = proj_blocks(t, qf_tiles[t])
                if t >= 2:
                    fillers = fillers + oproj_blocks(t - 2)
                attention(t - 1, qf_tiles[t - 1], fillers,
                          reserve=(3 if t >= 2 else 0), pre=6)
            attention(TC - 1, qf_tiles[TC - 1], oproj_blocks(TC - 2),
                      reserve=3, pre=6)
            # tail: o_proj of the last q-chunk, double-buffered across tags
            for i, blk in enumerate(oproj_blocks(TC - 1, tail=True)):
                blk(tag=("OP" if i % 2 == 0 else "A"))


def _build():
    global _NC
    if _NC is None:
        nc = bacc.Bacc("TRN2", target_bir_lowering=False, debug=False,
                       num_devices=8)
        _emit(nc)
        nc.compile()
        _NC = nc
    return _NC


def _prep_inputs(x, wq, bq, wk, bk, wv, bv, wo, bo, cos, sin):
    """Host-side shard + layout prep. Core c = (g, b): g = c % 4, b = c // 4."""
    inv_sqrt_d = 1.0 / math.sqrt(HD)
    f32 = np.float32
    bf16 = ml_dtypes.bfloat16
    cosT = np.ascontiguousarray(cos.T.astype(f32))
    sinT = sin.T.astype(f32)
    # rotate-half as one fused mul: row p of sinS holds the sin factor that
    # multiplies raw[p] before the half-swap writes it to partition p^64,
    # i.e. sinS[0:64] = sin_hi, sinS[64:128] = -sin_lo
    sinS = np.ascontiguousarray(
        np.concatenate([sinT[HD // 2:], -sinT[0:HD // 2]], axis=0))

    xTb = [np.ascontiguousarray(x[b].T.astype(bf16)) for b in range(B)]

    def pack(wT, nch):
        # [H or QD, M] -> [128, nch*M]: row p holds chunk-major slices
        m = wT.shape[1]
        return np.ascontiguousarray(
            wT.reshape(nch, 128, m).transpose(1, 0, 2).reshape(128, nch * m)
            .astype(bf16))

    in_maps = []
    for c in range(8):
        g, b = c % G, c // G
        wq_s = wq[QD * g:QD * (g + 1), :] * inv_sqrt_d
        bq_s = bq[QD * g:QD * (g + 1)] * inv_sqrt_d
        wk_s = wk[HD * g:HD * (g + 1), :]
        bk_s = bk[HD * g:HD * (g + 1)]
        wv_s = wv[HD * g:HD * (g + 1), :]
        bv_s = bv[HD * g:HD * (g + 1)]
        bias = np.zeros((128, 6), f32)
        bias[:, 0:4] = bq_s.reshape(GQ, HD).T
        bias[:, 4] = bk_s
        bias[:, 5] = bv_s
        wkvT = np.concatenate([wk_s.T, wv_s.T], axis=1)     # [H, 256]
        in_maps.append({
            "xT": xTb[b],
            "wqD": pack(wq_s.T, KC),
            "wkvD": pack(wkvT, KC),
            "woD": pack(wo[:, QD * g:QD * (g + 1)].T, GQ),
            "cosT": cosT,
            "sinS": sinS,
            "bqkv": bias,
            "onesd": np.ones((128, 128), bf16),
        })
    return in_maps


def run(inputs, trace=False):
    """Returns (full_output, BassKernelResults)."""
    inputs = {k: np.asarray(v) for k, v in inputs.items()}
    nc = _build()
    in_maps = _prep_inputs(**inputs)
    res = run_bass_kernel_spmd(nc, in_maps, core_ids=list(range(8)),
                               trace=trace)
    bo = inputs["bo"].astype(np.float64)
    out = np.empty((B, S, H), np.float32)
    for b in range(B):
        acc = np.zeros((S, H), np.float64)
        for g in range(G):
            acc += res.results[G * b + g]["outp"].astype(np.float64)
        out[b] = (acc + bo).astype(np.float32)
    return out, res


def kernel(**inputs):
    return run(inputs, trace=False)[0]
